# revision 35
# baseline (speedup 1.0000x reference)
"""Trainium2 Bass kernel for BCE + distance-decorrelation (DisCo) loss.

Reference math (N = 8192):
    bce  = mean((softplus(o) - o*l) * w)
    nw   = w * N / sum(w);  om = bf16(nw)  (weights used consistently)
    a_ij = |o_i - o_j|, b_ij = |e_i - e_j|  (from bf16-rounded o, e)
    u_i  = (1/N) sum_j om_j a_ij   (same v_i for b)
    num  = (1/N^2) sum_ij om_i om_j A_ij B_ij   (doubly-centered)
    disco = num / sqrt(den);  tot = bce + 0.1 * disco

Exact decomposition (as in the previous kernel):
    num*N^2  = T_ab + (2S-4N) P_uv + (4N^2-4NS+S^2) mA mB,  S = sum om
with u, v, P_*, mA, mB, T_aa, T_bb all O(N) host-side:
  - u_i = (1/N) sum_j om_j |o_i - o_j| computed EXACTLY on host via
    sort + prefix sums (O(N log N), float64).
  - T_aa, T_bb closed-form.
The ONLY O(N^2) quantity is T_ab = sum_ij om_i om_j a_ij b_ij. Using
|p| = 2 relu(p) - p for the SIGNED product p_ij = (o_i-o_j)(e_i-e_j)
(note a_ij b_ij = |p_ij|):
    T_ab = 2 R - CF,  R = sum_ij om_i om_j relu(p_ij)   (device)
           CF = sum_ij om_i om_j p_ij = 2[S sum(om o e) - sum(om o) sum(om e)]

Device computes ONLY R:
  - TensorE GENERATES q_ij = kappa om_i om_j p_ij tiles directly in PSUM
    as a rank-10 matmul: p is a rank-4 outer-product form
    (oe_i + oe_j - o_i e_j - o_j e_i); each om-scaled coefficient vector
    is split hi/lo in bf16 so every PE product is exact and q is accurate
    to ~1e-5 absolute (lo*lo cross terms dropped).
  - ACT/DVE do ONE fused pass per PSUM tile: relu + free-axis-reduce
    (activation accum_out / tensor_scalar op0=max, op1=add accum_out),
    alternating engines to split the load. No other elementwise work.
  - Symmetry: p_ij = p_ji, so only supertile pairs (i-chunk <= j-chunk)
    are generated; off-diagonal supertiles get kappa=2 folded into the
    (bf16-exact) stationary. i-chunks c and 15-c pair up so every core
    has EXACTLY 68 matmuls + 17 relu groups -> one SPMD program whose
    per-core differences live entirely in the DMAed data.

Host does the O(N) rest in float64: sorted row-sums, closed forms, BCE.
"""

from contextlib import ExitStack

import numpy as np
import ml_dtypes

import concourse.bacc as bacc
import concourse.bass as bass
import concourse.tile as tile
from concourse import mybir
from concourse.bass_utils import run_bass_kernel_spmd

N = 8192
NCORES = 8
CH = 512           # j-chunk width = moving cols per matmul = one PSUM bank
NCH = N // CH      # 16 i/j chunks
IB = 128           # i-block width (PSUM partitions)
NMM = 68           # matmuls per core (uniform across cores)
NGRP = 34          # relu groups of 2 matmuls ([128, 1024] PSUM each)
RANK = 10
LAM = 0.1

F32 = mybir.dt.float32
BF16 = mybir.dt.bfloat16
BF = ml_dtypes.bfloat16

HMM = NMM // 2          # 34 MMs per row-group lane
STAT_W = HMM * IB       # 4352
MOV_W = (HMM // 2) * CH  # 8704: consecutive lane-MM pairs share a j-chunk
INP_W = STAT_W + MOV_W  # 13056


def core_mms(c):
    """68 (i_block, j_chunk, kappa) triples for core c: i-chunks {c, 15-c},
    j-chunks >= i-chunk (triangle), kappa=2 off-diagonal. J-major order so
    each parity lane sees runs of 2 equal-J matmuls (mov chunk shared)."""
    mms = []
    for Cc in (c, NCH - 1 - c):
        for J in range(Cc, NCH):
            for b in range(4 * Cc, 4 * Cc + 4):
                mms.append((b, J, 1.0 if J == Cc else 2.0))
    assert len(mms) == NMM
    return mms


def build_program():
    nc = bacc.Bacc(None)
    # even/odd matmuls draw stationaries+moving from PE row-group 0 / 1
    # (SBUF partitions 0-9 / 32-41) so each LDWEIGHTS overlaps the previous
    # matmul instead of serializing on the same row-group.
    inpA = nc.dram_tensor("inpA", [RANK, INP_W], BF16, kind="ExternalInput")
    inpB = nc.dram_tensor("inpB", [RANK, INP_W], BF16, kind="ExternalInput")
    slots_out = nc.dram_tensor("slots", [128, NGRP], F32, kind="ExternalOutput")
    NACT = len(set(range(0, NGRP, 2)))  # ACT slot count (first NACT slot cols)

    with tile.TileContext(nc) as tc, ExitStack() as ctx:
        const = ctx.enter_context(tc.tile_pool(name="const", bufs=1))
        ps = ctx.enter_context(tc.tile_pool(name="ps", bufs=4, space="PSUM"))
        outp = ctx.enter_context(tc.tile_pool(name="outp", bufs=1))

        # per-DMA-slice tiles with chunk-aligned boundaries, so each matmul
        # waits only on the one DMA carrying its data (slice 0 = all
        # stationaries + first two mov chunks -> first matmuls start early)
        SLW = [STAT_W + 2 * CH, 7 * CH, 8 * CH]
        SLO = [0, SLW[0], SLW[0] + SLW[1]]
        tA, tB = [], []
        for q in range(3):
            tA.append(const.tile([RANK, SLW[q]], BF16, tag=f"tA{q}", name=f"tA{q}"))
            tB.append(const.tile([32 + RANK, SLW[q]], BF16, tag=f"tB{q}", name=f"tB{q}"))
        for q in range(3):
            sl = slice(SLO[q], SLO[q] + SLW[q])
            nc.sync.dma_start(out=tA[q][:, :], in_=inpA[:, sl])
            nc.sync.dma_start(out=tB[q][32:32 + RANK, :], in_=inpB[:, sl])

        def mm_aps(t):
            m = t // 2           # lane MM index (stat block)
            r = m // 2           # mov run index (shared j-chunk)
            tiles = tA if t % 2 == 0 else tB
            rows = slice(0, RANK) if t % 2 == 0 else slice(32, 32 + RANK)
            stat_ap = tiles[0][rows, m * IB:(m + 1) * IB]
            if r < 2:
                mv = tiles[0][rows, STAT_W + r * CH:STAT_W + (r + 1) * CH]
            elif r < 9:
                mv = tiles[1][rows, (r - 2) * CH:(r - 1) * CH]
            else:
                mv = tiles[2][rows, (r - 9) * CH:(r - 8) * CH]
            return stat_ap, mv

        # separate slot tiles per engine: each engine's slots DMA out as soon
        # as that engine finishes, overlapping the other's tail
        slots_a = outp.tile([128, NACT], F32, tag="slots_a")
        slots_v = outp.tile([128, NGRP - NACT], F32, tag="slots_v")
        dummy = outp.tile([128, 1], F32, tag="dummy")

        act_set = set(range(0, NGRP, 2))
        na = nv = 0
        for g in range(NGRP):
            # static PSUM ownership: ACT rotates tiles {0,1}, DVE {2,3} --
            # the two consumer chains never share a buffer, so each engine
            # only ever waits on its own previous op
            eng_act = g in act_set
            if eng_act:
                tag = f"qa{na % 2}"
                acc = slots_a[:, na:na + 1]
                na += 1
            else:
                tag = f"qv{nv % 2}"
                acc = slots_v[:, nv:nv + 1]
                nv += 1
            qt = ps.tile([128, 2 * CH], F32, tag=tag, bufs=1, name=f"q{g}")
            for s in range(2):
                stat_ap, mov_ap = mm_aps(2 * g + s)
                nc.tensor.matmul(
                    qt[:, s * CH:(s + 1) * CH],
                    stat_ap,
                    mov_ap,
                    start=True,
                    stop=True,
                )
            if eng_act:
                # ACT: out = Relu(in), accum_out = sum over free axis
                nc.scalar.activation(
                    out=qt, in_=qt,
                    func=mybir.ActivationFunctionType.Relu,
                    accum_out=acc,
                )
            else:
                # DVE: out = max(in, 0), accum_out = add-reduce(out)
                nc.vector.tensor_scalar(
                    out=dummy.broadcast_to(qt.shape),
                    in0=qt,
                    scalar1=0.0,
                    scalar2=None,
                    op0=mybir.AluOpType.max,
                    op1=mybir.AluOpType.add,
                    accum_out=acc,
                )
        nc.sync.dma_start(out=slots_out[:, 0:NACT], in_=slots_a)
        nc.sync.dma_start(out=slots_out[:, NACT:NGRP], in_=slots_v)

    nc.finalize()
    return nc


def _hilo(x64):
    hi = x64.astype(np.float32).astype(BF).astype(np.float64)
    lo = (x64 - hi).astype(np.float32).astype(BF).astype(np.float64)
    return hi, lo


def make_in_maps(ob, eb, om):
    """Per-core [RANK, INP_W] bf16 input: per-MM stationary blocks (kappa
    folded in) then per-MM moving blocks. ob/eb/om are f64 of bf16 values."""
    A_hi, A_lo = _hilo(om * ob * eb)
    C_hi, C_lo = _hilo(om * ob)
    D_hi, D_lo = _hilo(om * eb)
    ones = np.ones(N)
    # rank-10: q = sum_r stat_r[i] * mov_r[j]
    stat_full = np.stack([A_hi, A_lo, om, om, -C_hi, -C_hi, -C_lo, -D_hi, -D_hi, -D_lo])
    mov_full = np.stack([om, om, A_hi, A_lo, D_hi, D_lo, D_hi, C_hi, C_lo, C_hi])
    stat_full = stat_full.astype(np.float32)
    mov_full16 = mov_full.astype(np.float32).astype(BF)

    in_maps = []
    for c in range(NCORES):
        bufA = np.empty((RANK, INP_W), dtype=BF)
        bufB = np.empty((RANK, INP_W), dtype=BF)
        for t, (b, J, k) in enumerate(core_mms(c)):
            st = stat_full[:, b * IB:(b + 1) * IB]
            if k != 1.0:
                st = st * np.float32(2.0)  # exact in bf16
            buf = bufA if t % 2 == 0 else bufB
            m = t // 2
            r = m // 2
            buf[:, m * IB:(m + 1) * IB] = st.astype(BF)
            buf[:, STAT_W + r * CH:STAT_W + (r + 1) * CH] = mov_full16[:, J * CH:(J + 1) * CH]
        in_maps.append({"inpA": bufA, "inpB": bufB})
    return in_maps


def _rowsums_sorted(x, omw):
    """sum_j om_j |x_i - x_j| exactly via sort + prefix sums (f64)."""
    idx = np.argsort(x, kind="stable")
    xs, oms = x[idx], omw[idx]
    W, C = np.cumsum(oms), np.cumsum(oms * xs)
    su_sorted = xs * W - C + (C[-1] - C) - xs * (W[-1] - W)
    su = np.empty_like(su_sorted)
    su[idx] = su_sorted
    return su


def combine(results, o64, l64, e64, w64, ob, eb, om):
    R = sum(float(results[c]["slots"].astype(np.float64).sum()) for c in range(NCORES))
    S = om.sum()
    CF = 2.0 * (S * (om * ob * eb).sum() - (om * ob).sum() * (om * eb).sum())
    T_ab = 2.0 * R - CF

    u = _rowsums_sorted(ob, om) / N
    v = _rowsums_sorted(eb, om) / N
    P_uv, P_uu, P_vv = (om * u * v).sum(), (om * u * u).sum(), (om * v * v).sum()
    mA, mB = (om * u).sum() / N, (om * v).sum() / N
    T_aa = 2 * S * (om * ob * ob).sum() - 2 * (om * ob).sum() ** 2
    T_bb = 2 * S * (om * eb * eb).sum() - 2 * (om * eb).sum() ** 2
    c1, c2 = 2 * S - 4 * N, 4 * N * N - 4 * N * S + S * S
    num = (T_ab + c1 * P_uv + c2 * mA * mB) / N**2
    denA = (T_aa + c1 * P_uu + c2 * mA * mA) / N**2
    denB = (T_bb + c1 * P_vv + c2 * mB * mB) / N**2
    disco = num / np.sqrt(denA * denB)

    bce = float(np.mean((np.logaddexp(0.0, o64) - o64 * l64) * w64))
    tot = bce + LAM * disco
    return (np.float32(bce), np.float32(disco), np.float32(tot))


def run(outputs, labels, event, weights, **spmd_kwargs):
    o64 = np.asarray(outputs, dtype=np.float64)
    l64 = np.asarray(labels, dtype=np.float64)
    e64 = np.asarray(event, dtype=np.float64)
    w64 = np.asarray(weights, dtype=np.float64)
    assert o64.shape == (N,)

    # normalized weights, mimicking the reference's f32 computation, then bf16
    nw = (w64.astype(np.float32) * np.float32(N)
          / w64.astype(np.float32).sum(dtype=np.float32)).astype(np.float32)
    om = nw.astype(BF).astype(np.float64)
    ob = o64.astype(np.float32).astype(BF).astype(np.float64)
    eb = e64.astype(np.float32).astype(BF).astype(np.float64)

    nc = build_program()
    in_maps = make_in_maps(ob, eb, om)
    bkr = run_bass_kernel_spmd(nc, in_maps, list(range(NCORES)), **spmd_kwargs)
    return combine(bkr.results, o64, l64, e64, w64, ob, eb, om), bkr


def kernel(outputs, labels, event, weights):
    out, _ = run(outputs, labels, event, weights)
    return out


# revision 36
# speedup vs baseline: 1.1768x; 1.1768x over previous
"""Trainium2 Bass kernel for BCE + distance-decorrelation (DisCo) loss.

Reference math (N = 8192):
    bce  = mean((softplus(o) - o*l) * w)
    nw   = w * N / sum(w);  om = bf16(nw)  (weights used consistently)
    a_ij = |o_i - o_j|, b_ij = |e_i - e_j|  (from bf16-rounded o, e)
    u_i  = (1/N) sum_j om_j a_ij   (same v_i for b)
    num  = (1/N^2) sum_ij om_i om_j A_ij B_ij   (doubly-centered)
    disco = num / sqrt(den);  tot = bce + 0.1 * disco

Exact decomposition (as in the previous kernel):
    num*N^2  = T_ab + (2S-4N) P_uv + (4N^2-4NS+S^2) mA mB,  S = sum om
with u, v, P_*, mA, mB, T_aa, T_bb all O(N) host-side:
  - u_i = (1/N) sum_j om_j |o_i - o_j| computed EXACTLY on host via
    sort + prefix sums (O(N log N), float64).
  - T_aa, T_bb closed-form.
The ONLY O(N^2) quantity is T_ab = sum_ij om_i om_j a_ij b_ij. Using
|p| = 2 relu(p) - p for the SIGNED product p_ij = (o_i-o_j)(e_i-e_j)
(note a_ij b_ij = |p_ij|):
    T_ab = 2 R - CF,  R = sum_ij om_i om_j relu(p_ij)   (device)
           CF = sum_ij om_i om_j p_ij = 2[S sum(om o e) - sum(om o) sum(om e)]

Device computes ONLY R:
  - TensorE GENERATES q_ij = kappa om_i om_j p_ij tiles directly in PSUM
    as a rank-10 matmul: p is a rank-4 outer-product form
    (oe_i + oe_j - o_i e_j - o_j e_i); each om-scaled coefficient vector
    is split hi/lo in bf16 so every PE product is exact and q is accurate
    to ~1e-5 absolute (lo*lo cross terms dropped).
  - ACT/DVE do ONE fused pass per PSUM tile: relu + free-axis-reduce
    (activation accum_out / tensor_scalar op0=max, op1=add accum_out),
    alternating engines to split the load. No other elementwise work.
  - Symmetry: p_ij = p_ji, so only supertile pairs (i-chunk <= j-chunk)
    are generated; off-diagonal supertiles get kappa=2 folded into the
    (bf16-exact) stationary. i-chunks c and 15-c pair up so every core
    has EXACTLY 68 matmuls + 17 relu groups -> one SPMD program whose
    per-core differences live entirely in the DMAed data.

Host does the O(N) rest in float64: sorted row-sums, closed forms, BCE.
"""

from contextlib import ExitStack

import numpy as np
import ml_dtypes

import concourse.bacc as bacc
import concourse.bass as bass
import concourse.tile as tile
from concourse import mybir
from concourse.bass_utils import run_bass_kernel_spmd

N = 8192
NCORES = 8
CH = 512           # j-chunk width = moving cols per matmul = one PSUM bank
NCH = N // CH      # 16 i/j chunks
IB = 128           # i-block width (PSUM partitions)
NMM = 68           # matmuls per core (uniform across cores)
NGRP = 34          # relu groups of 2 matmuls ([128, 1024] PSUM each)
RANK = 10
LAM = 0.1

F32 = mybir.dt.float32
BF16 = mybir.dt.bfloat16
BF = ml_dtypes.bfloat16

HMM = NMM // 2          # 34 MMs per row-group lane
STAT_W = HMM * IB       # 4352
MOV_W = (HMM // 2) * CH  # 8704: consecutive lane-MM pairs share a j-chunk
INP_W = STAT_W + MOV_W  # 13056


def core_mms(c):
    """68 (i_block, j_chunk, kappa) triples for core c: i-chunks {c, 15-c},
    j-chunks >= i-chunk (triangle), kappa=2 off-diagonal. J-major order so
    each parity lane sees runs of 2 equal-J matmuls (mov chunk shared)."""
    mms = []
    for Cc in (c, NCH - 1 - c):
        for J in range(Cc, NCH):
            for b in range(4 * Cc, 4 * Cc + 4):
                mms.append((b, J, 1.0 if J == Cc else 2.0))
    assert len(mms) == NMM
    return mms


def build_program():
    nc = bacc.Bacc(None)
    # even/odd matmuls draw stationaries+moving from PE row-group 0 / 1
    # (SBUF partitions 0-9 / 32-41) so each LDWEIGHTS overlaps the previous
    # matmul instead of serializing on the same row-group.
    inpA = nc.dram_tensor("inpA", [RANK, INP_W], BF16, kind="ExternalInput")
    inpB = nc.dram_tensor("inpB", [RANK, INP_W], BF16, kind="ExternalInput")
    slots_out = nc.dram_tensor("slots", [128, NGRP], F32, kind="ExternalOutput")
    NACT = len(set(range(0, NGRP, 2)))  # ACT slot count (first NACT slot cols)

    with tile.TileContext(nc) as tc, ExitStack() as ctx:
        const = ctx.enter_context(tc.tile_pool(name="const", bufs=1))
        ps = ctx.enter_context(tc.tile_pool(name="ps", bufs=4, space="PSUM"))
        outp = ctx.enter_context(tc.tile_pool(name="outp", bufs=1))

        # per-DMA-slice tiles with chunk-aligned boundaries, so each matmul
        # waits only on the one DMA carrying its data (slice 0 = all
        # stationaries + first two mov chunks -> first matmuls start early)
        SLW = [STAT_W + 2 * CH, 15 * CH]
        SLO = [0, SLW[0]]
        tA, tB = [], []
        for q in range(2):
            tA.append(const.tile([RANK, SLW[q]], BF16, tag=f"tA{q}", name=f"tA{q}"))
            tB.append(const.tile([32 + RANK, SLW[q]], BF16, tag=f"tB{q}", name=f"tB{q}"))
        for q in range(2):
            sl = slice(SLO[q], SLO[q] + SLW[q])
            nc.sync.dma_start(out=tA[q][:, :], in_=inpA[:, sl])
            nc.sync.dma_start(out=tB[q][32:32 + RANK, :], in_=inpB[:, sl])

        def mm_aps(t):
            m = t // 2           # lane MM index (stat block)
            r = m // 2           # mov run index (shared j-chunk)
            tiles = tA if t % 2 == 0 else tB
            rows = slice(0, RANK) if t % 2 == 0 else slice(32, 32 + RANK)
            stat_ap = tiles[0][rows, m * IB:(m + 1) * IB]
            if r < 2:
                mv = tiles[0][rows, STAT_W + r * CH:STAT_W + (r + 1) * CH]
            else:
                mv = tiles[1][rows, (r - 2) * CH:(r - 1) * CH]
            return stat_ap, mv

        # separate slot tiles per engine: each engine's slots DMA out as soon
        # as that engine finishes, overlapping the other's tail
        slots_a = outp.tile([128, NACT], F32, tag="slots_a")
        slots_v = outp.tile([128, NGRP - NACT], F32, tag="slots_v")
        dummy = outp.tile([128, 1], F32, tag="dummy")

        act_set = set(range(0, NGRP, 2))
        na = nv = 0
        for g in range(NGRP):
            # static PSUM ownership: ACT rotates tiles {0,1}, DVE {2,3} --
            # the two consumer chains never share a buffer, so each engine
            # only ever waits on its own previous op
            eng_act = g in act_set
            if eng_act:
                tag = f"qa{na % 2}"
                acc = slots_a[:, na:na + 1]
                na += 1
            else:
                tag = f"qv{nv % 2}"
                acc = slots_v[:, nv:nv + 1]
                nv += 1
            qt = ps.tile([128, 2 * CH], F32, tag=tag, bufs=1, name=f"q{g}")
            for s in range(2):
                stat_ap, mov_ap = mm_aps(2 * g + s)
                nc.tensor.matmul(
                    qt[:, s * CH:(s + 1) * CH],
                    stat_ap,
                    mov_ap,
                    start=True,
                    stop=True,
                )
            if eng_act:
                # ACT: out = Relu(in), accum_out = sum over free axis
                nc.scalar.activation(
                    out=qt, in_=qt,
                    func=mybir.ActivationFunctionType.Relu,
                    accum_out=acc,
                )
            else:
                # DVE: out = max(in, 0), accum_out = add-reduce(out)
                nc.vector.tensor_scalar(
                    out=dummy.broadcast_to(qt.shape),
                    in0=qt,
                    scalar1=0.0,
                    scalar2=None,
                    op0=mybir.AluOpType.max,
                    op1=mybir.AluOpType.add,
                    accum_out=acc,
                )
        nc.sync.dma_start(out=slots_out[:, 0:NACT], in_=slots_a)
        nc.sync.dma_start(out=slots_out[:, NACT:NGRP], in_=slots_v)

    nc.finalize()
    return nc


def _hilo(x64):
    hi = x64.astype(np.float32).astype(BF).astype(np.float64)
    lo = (x64 - hi).astype(np.float32).astype(BF).astype(np.float64)
    return hi, lo


def make_in_maps(ob, eb, om):
    """Per-core [RANK, INP_W] bf16 input: per-MM stationary blocks (kappa
    folded in) then per-MM moving blocks. ob/eb/om are f64 of bf16 values."""
    A_hi, A_lo = _hilo(om * ob * eb)
    C_hi, C_lo = _hilo(om * ob)
    D_hi, D_lo = _hilo(om * eb)
    ones = np.ones(N)
    # rank-10: q = sum_r stat_r[i] * mov_r[j]
    stat_full = np.stack([A_hi, A_lo, om, om, -C_hi, -C_hi, -C_lo, -D_hi, -D_hi, -D_lo])
    mov_full = np.stack([om, om, A_hi, A_lo, D_hi, D_lo, D_hi, C_hi, C_lo, C_hi])
    stat_full = stat_full.astype(np.float32)
    mov_full16 = mov_full.astype(np.float32).astype(BF)

    in_maps = []
    for c in range(NCORES):
        bufA = np.empty((RANK, INP_W), dtype=BF)
        bufB = np.empty((RANK, INP_W), dtype=BF)
        for t, (b, J, k) in enumerate(core_mms(c)):
            st = stat_full[:, b * IB:(b + 1) * IB]
            if k != 1.0:
                st = st * np.float32(2.0)  # exact in bf16
            buf = bufA if t % 2 == 0 else bufB
            m = t // 2
            r = m // 2
            buf[:, m * IB:(m + 1) * IB] = st.astype(BF)
            buf[:, STAT_W + r * CH:STAT_W + (r + 1) * CH] = mov_full16[:, J * CH:(J + 1) * CH]
        in_maps.append({"inpA": bufA, "inpB": bufB})
    return in_maps


def _rowsums_sorted(x, omw):
    """sum_j om_j |x_i - x_j| exactly via sort + prefix sums (f64)."""
    idx = np.argsort(x, kind="stable")
    xs, oms = x[idx], omw[idx]
    W, C = np.cumsum(oms), np.cumsum(oms * xs)
    su_sorted = xs * W - C + (C[-1] - C) - xs * (W[-1] - W)
    su = np.empty_like(su_sorted)
    su[idx] = su_sorted
    return su


def combine(results, o64, l64, e64, w64, ob, eb, om):
    R = sum(float(results[c]["slots"].astype(np.float64).sum()) for c in range(NCORES))
    S = om.sum()
    CF = 2.0 * (S * (om * ob * eb).sum() - (om * ob).sum() * (om * eb).sum())
    T_ab = 2.0 * R - CF

    u = _rowsums_sorted(ob, om) / N
    v = _rowsums_sorted(eb, om) / N
    P_uv, P_uu, P_vv = (om * u * v).sum(), (om * u * u).sum(), (om * v * v).sum()
    mA, mB = (om * u).sum() / N, (om * v).sum() / N
    T_aa = 2 * S * (om * ob * ob).sum() - 2 * (om * ob).sum() ** 2
    T_bb = 2 * S * (om * eb * eb).sum() - 2 * (om * eb).sum() ** 2
    c1, c2 = 2 * S - 4 * N, 4 * N * N - 4 * N * S + S * S
    num = (T_ab + c1 * P_uv + c2 * mA * mB) / N**2
    denA = (T_aa + c1 * P_uu + c2 * mA * mA) / N**2
    denB = (T_bb + c1 * P_vv + c2 * mB * mB) / N**2
    disco = num / np.sqrt(denA * denB)

    bce = float(np.mean((np.logaddexp(0.0, o64) - o64 * l64) * w64))
    tot = bce + LAM * disco
    return (np.float32(bce), np.float32(disco), np.float32(tot))


def run(outputs, labels, event, weights, **spmd_kwargs):
    o64 = np.asarray(outputs, dtype=np.float64)
    l64 = np.asarray(labels, dtype=np.float64)
    e64 = np.asarray(event, dtype=np.float64)
    w64 = np.asarray(weights, dtype=np.float64)
    assert o64.shape == (N,)

    # normalized weights, mimicking the reference's f32 computation, then bf16
    nw = (w64.astype(np.float32) * np.float32(N)
          / w64.astype(np.float32).sum(dtype=np.float32)).astype(np.float32)
    om = nw.astype(BF).astype(np.float64)
    ob = o64.astype(np.float32).astype(BF).astype(np.float64)
    eb = e64.astype(np.float32).astype(BF).astype(np.float64)

    nc = build_program()
    in_maps = make_in_maps(ob, eb, om)
    bkr = run_bass_kernel_spmd(nc, in_maps, list(range(NCORES)), **spmd_kwargs)
    return combine(bkr.results, o64, l64, e64, w64, ob, eb, om), bkr


def kernel(outputs, labels, event, weights):
    out, _ = run(outputs, labels, event, weights)
    return out
